# revision 1
# baseline (speedup 1.0000x reference)
"""Trainium2 kernel for nn_AxisFuserLayer (full HW implementation, 8 cores).

Phase A (data-parallel over batch): core c runs batch c's embed + mamba
(selective scan via DVE tensor_tensor_scan, exploiting A[d,s] = -(s+1)) + LN.
Exchange: AllToAll of the LN'd mamba branch (bf16, 128-position slices).
Phase B (position-parallel): core c runs the mis-batched attention (softmax
over the 8 batch elements at each position) for its 128-position slice of all
3 branches (x, acc-mamba, ang), for all batches, plus the output projection.

Host does all transposes/packing (free); HW layouts are channel-major.
"""

import numpy as np
import ml_dtypes

try:        # persistent jax/XLA cache: lets a fresh process reuse the compiled NEFF
    import jax
    jax.config.update("jax_compilation_cache_dir", "/tmp/jax_bass_cache")
    jax.config.update("jax_persistent_cache_min_compile_time_secs", 0.0)
except Exception:
    pass

B, L, DM, NH = 8, 1024, 256, 8
DI, DS, DC, DTR = 512, 16, 4, 16
DH = DM // NH          # 32
SL = L // 8            # 128 positions per core per branch
NPOS = 3 * SL          # 384 positions per core
FAT = B * NPOS         # 3072 attention free size  (beta-major: col = b*128+n within beta*1024)

BF = ml_dtypes.bfloat16


# ---------------------------------------------------------------- weight packing
def _pack_specs():
    """(name, partitions, free_cols): wb = critical phase-A weights (first DMA),
    wb2 = phase-B weights (second DMA), wf = f32 params."""
    wb = [
        ("acc_wT", 12, 256),
        ("inw0", 128, 1024), ("inw1", 128, 1024),
        ("xw0", 128, 48), ("xw1", 128, 48), ("xw2", 128, 48), ("xw3", 128, 48),
        ("dtw", 16, 512),
        ("ones_col", 128, 1), ("ident", 128, 128),
    ]
    for j in range(DC):
        for db in range(4):
            wb.append((f"cd{j}_{db}", 128, 128))
    wb2 = [
        ("ang_wT", 12, 256),
        ("ow0", 128, 256), ("ow1", 128, 256), ("ow2", 128, 256), ("ow3", 128, 256),
        ("aiw0", 128, 768), ("aiw1", 128, 768),
        ("aow0", 128, 256), ("aow1", 128, 256),
        ("hsel0", 128, 8), ("hsel1", 128, 8),
        ("ind0", 8, 128), ("ind1", 8, 128),
    ]
    wf = [("acc_b0", 128, 1), ("acc_b1", 128, 1), ("ang_b0", 128, 1), ("ang_b1", 128, 1),
          ("ones_row_f", 1, 128)]
    for db in range(4):
        wf += [(f"cb{db}", 128, 1), (f"dtb{db}", 128, 1), (f"dp{db}", 128, 1)]
    for j in range(DC):
        for db in range(4):
            wf.append((f"cw{j}_{db}", 128, 1))
    for i in range(3):          # norms: 0=x, 1=acc, 2=ang
        for pb in range(2):
            wf += [(f"lnw{i}{pb}", 128, 1), (f"lnb{i}{pb}", 128, 1)]
    for mb in range(6):
        wf.append((f"aib{mb}", 128, 1))
    for mb in range(2):
        wf.append((f"aob{mb}", 128, 1))

    def offsets(spec):
        offs, o = {}, 0
        for nm, p, f in spec:
            offs[nm] = (o, p, f)
            o += f
        return offs, o

    wb_offs, wb_tot = offsets(wb)
    wb2_offs, wb2_tot = offsets(wb2)
    wf_offs, wf_tot = offsets(wf)
    return wb_offs, wb_tot, wb2_offs, wb2_tot, wf_offs, wf_tot


WB_OFFS, WB_TOT, WB2_OFFS, WB2_TOT, WF_OFFS, WF_TOT = _pack_specs()


def _pack_weights(w):
    wb = np.zeros((128, WB_TOT), dtype=BF)
    wb2 = np.zeros((128, WB2_TOT), dtype=BF)
    wf = np.zeros((128, WF_TOT), dtype=np.float32)

    def putb(nm, arr):
        if nm in WB_OFFS:
            o, p, f = WB_OFFS[nm]
            dst = wb
        else:
            o, p, f = WB2_OFFS[nm]
            dst = wb2
        dst[0:p, o:o + f] = np.asarray(arr, np.float32).reshape(p, f).astype(BF)

    def putf(nm, arr):
        o, p, f = WF_OFFS[nm]
        wf[0:p, o:o + f] = np.asarray(arr, np.float32).reshape(p, f)

    putb("acc_wT", w["acc_w"].T)
    putb("ang_wT", w["ang_w"].T)
    inw = w["in_proj_w"].T                      # (256, 1024)
    putb("inw0", inw[0:128]); putb("inw1", inw[128:256])
    xw = w["x_proj_w"].T                        # (512, 48)
    for i in range(4):
        putb(f"xw{i}", xw[i * 128:(i + 1) * 128])
    putb("dtw", w["dt_proj_w"].T)               # (16, 512)
    ow = w["out_proj_w"].T                      # (512, 256)
    for i in range(4):
        putb(f"ow{i}", ow[i * 128:(i + 1) * 128])
    aiw = w["attn_in_w"].T.copy()               # (256, 768)
    aiw[:, 0:DM] *= 1.0 / np.sqrt(DH)           # fold q scaling
    putb("aiw0", aiw[0:128]); putb("aiw1", aiw[128:256])
    aow = w["attn_out_w"].T                     # (256, 256)
    putb("aow0", aow[0:128]); putb("aow1", aow[128:256])
    for pb in range(2):
        hs = np.zeros((128, 8), np.float32)
        for p in range(128):
            hs[p, 4 * pb + p // 32] = 1.0
        putb(f"hsel{pb}", hs)
    for pb in range(2):
        ind = np.zeros((8, 128), np.float32)
        for p in range(128):
            ind[4 * pb + p // 32, p] = 1.0
        putb(f"ind{pb}", ind)
    putb("ones_col", np.ones((128, 1), np.float32))
    putb("ident", np.eye(128, dtype=np.float32))
    conv_w = np.ascontiguousarray(w["conv_w"][:, 0, :])     # (DI, DC)
    for j in range(DC):
        for db in range(4):
            d = np.zeros((128, 128), np.float32)
            np.fill_diagonal(d, conv_w[db * 128:(db + 1) * 128, j])
            putb(f"cd{j}_{db}", d)
            putf(f"cw{j}_{db}", conv_w[db * 128:(db + 1) * 128, j:j + 1])

    putf("ones_row_f", np.ones((1, 128), np.float32))
    putf("acc_b0", w["acc_b"][0:128, None]); putf("acc_b1", w["acc_b"][128:256, None])
    putf("ang_b0", w["ang_b"][0:128, None]); putf("ang_b1", w["ang_b"][128:256, None])
    for db in range(4):
        putf(f"cb{db}", w["conv_b"][db * 128:(db + 1) * 128, None])
        putf(f"dtb{db}", w["dt_proj_b"][db * 128:(db + 1) * 128, None])
        putf(f"dp{db}", w["Dp"][db * 128:(db + 1) * 128, None])
    lnw = [w["norm_w"], w["norm_acc_w"], w["norm_ang_w"]]
    lnb = [w["norm_b"], w["norm_acc_b"], w["norm_ang_b"]]
    for i in range(3):
        for pb in range(2):
            putf(f"lnw{i}{pb}", lnw[i][pb * 128:(pb + 1) * 128, None])
            putf(f"lnb{i}{pb}", lnb[i][pb * 128:(pb + 1) * 128, None])
    for mb in range(6):
        putf(f"aib{mb}", w["attn_in_b"][mb * 128:(mb + 1) * 128, None])
    for mb in range(2):
        putf(f"aob{mb}", w["attn_out_b"][mb * 128:(mb + 1) * 128, None])
    return wb, wb2, wf


# ---------------------------------------------------------------- bass program
_NC_CACHE = {}


def _build(debug=False):
    import concourse.bacc as bacc
    import concourse.tile as tile
    from concourse import mybir
    from contextlib import ExitStack

    f32 = mybir.dt.float32
    bf16 = mybir.dt.bfloat16
    AF = mybir.ActivationFunctionType
    OP = mybir.AluOpType

    nc = bacc.Bacc(num_devices=B)

    wb_d = nc.dram_tensor("wb", (128, WB_TOT), bf16, kind="ExternalInput")
    wb2_d = nc.dram_tensor("wb2", (128, WB2_TOT), bf16, kind="ExternalInput")
    wf_d = nc.dram_tensor("wf", (128, WF_TOT), f32, kind="ExternalInput")
    accT_d = nc.dram_tensor("accT", (12, L), bf16, kind="ExternalInput")
    angT_d = nc.dram_tensor("angT", (12, L), bf16, kind="ExternalInput")
    xs_d = nc.dram_tensor("xs", (256, L), bf16, kind="ExternalInput")
    out_d = nc.dram_tensor("out", (256, B, 3, SL), f32, kind="ExternalOutput")
    if debug:
        dbg_d = nc.dram_tensor("dbg", (2, 128, L), f32, kind="ExternalOutput")

    NT = 2          # L // 512

    with ExitStack() as ctx:
        tc = ctx.enter_context(tile.TileContext(nc))
        const = ctx.enter_context(tc.tile_pool(name="const", bufs=1))
        sb = ctx.enter_context(tc.tile_pool(name="sb", bufs=1))
        scr = ctx.enter_context(tc.tile_pool(name="scr", bufs=2))
        psA = ctx.enter_context(tc.tile_pool(name="psA", bufs=2, space="PSUM"))
        psS = ctx.enter_context(tc.tile_pool(name="psS", bufs=3, space="PSUM"))
        dram = ctx.enter_context(tc.tile_pool(name="dram", bufs=1, space="DRAM"))

        wbt = const.tile([128, WB_TOT], bf16, tag="wb")
        nc.sync.dma_start(out=wbt[:], in_=wb_d[:, :])
        wft = const.tile([128, WF_TOT], f32, tag="wf")
        nc.sync.dma_start(out=wft[:], in_=wf_d[:, :])
        wbt2 = const.tile([128, WB2_TOT], bf16, tag="wb2")
        nc.sync.dma_start(out=wbt2[:], in_=wb2_d[:, :])

        def VB(nm):
            if nm in WB_OFFS:
                o, p, f = WB_OFFS[nm]
                return wbt[0:p, o:o + f]
            o, p, f = WB2_OFFS[nm]
            return wbt2[0:p, o:o + f]

        def VF(nm):
            o, p, f = WF_OFFS[nm]
            return wft[0:p, o:o + f]

        def nsl(t, n, w=512):
            return t[:, n * w:(n + 1) * w]

        eps_t = const.tile([1, 1], f32, tag="eps")
        nc.vector.memset(eps_t[:], 1e-5)

        # ---------------- phase A: embed acc -> mamba -> LN -> exchange
        accT = sb.tile([12, L], bf16, tag="accT")
        nc.sync.dma_start(out=accT[:], in_=accT_d[:, :])

        acc_emb = []
        for pb in range(2):
            t = sb.tile([128, L], bf16, tag=f"accemb{pb}")
            for n in range(NT):
                p = psA.tile([128, 512], f32, tag="mm", name="p_emb")
                nc.tensor.matmul(p[:], VB("acc_wT")[:, pb * 128:(pb + 1) * 128],
                                 nsl(accT, n), start=True, stop=True)
                nc.scalar.activation(nsl(t, n), p[:], AF.Identity,
                                     bias=VF(f"acc_b{pb}")[:, 0:1], scale=1.0)
            acc_emb.append(t)

        # in_proj -> xi (4), z (4)
        xi_t, z_t = [], []
        for mb in range(8):
            t = sb.tile([128, L], bf16, tag=f"xz{mb}")
            for n in range(NT):
                p = psA.tile([128, 512], f32, tag="mm", name="p_inp")
                for kb in range(2):
                    nc.tensor.matmul(p[:], VB(f"inw{kb}")[:, mb * 128:(mb + 1) * 128],
                                     nsl(acc_emb[kb], n), start=(kb == 0), stop=(kb == 1))
                if mb < 4:
                    nc.vector.tensor_copy(nsl(t, n), p[:])
                else:
                    nc.scalar.activation(nsl(t, n), p[:], AF.Silu, bias=0.0, scale=1.0)
            (xi_t if mb < 4 else z_t).append(t)

        # causal depthwise conv + silu -> xc
        xc_t = []
        for db in range(4):
            xc = sb.tile([128, L], bf16, tag=f"xc{db}")
            for n in range(NT):
                p = psA.tile([128, 512], f32, tag="mm", name="p_cv")
                nc.tensor.matmul(p[:], VB(f"cd3_{db}")[:], nsl(xi_t[db], n),
                                 start=True, stop=False)
                for j in range(DC - 1):
                    sh = DC - 1 - j
                    if n == 0:
                        nc.tensor.matmul(p[:, sh:], VB(f"cd{j}_{db}")[:],
                                         xi_t[db][:, 0:512 - sh],
                                         start=False, stop=(j == DC - 2))
                    else:
                        nc.tensor.matmul(p[:], VB(f"cd{j}_{db}")[:],
                                         xi_t[db][:, n * 512 - sh:(n + 1) * 512 - sh],
                                         start=False, stop=(j == DC - 2))
                nc.scalar.activation(nsl(xc, n), p[:], AF.Silu,
                                     bias=VF(f"cb{db}")[:, 0:1], scale=1.0)
            xc_t.append(xc)

        # x_proj -> dt (16, L), bc (32, L)
        dt_sb = sb.tile([16, L], bf16, tag="dtS")
        bc_sb = sb.tile([32, L], bf16, tag="bcS")
        for n in range(NT):
            p = psA.tile([128, 512], f32, tag="mm", name="p_xp")
            for kb in range(4):
                nc.tensor.matmul(p[0:16, :], VB(f"xw{kb}")[:, 0:16], nsl(xc_t[kb], n),
                                 start=(kb == 0), stop=(kb == 3))
                nc.tensor.matmul(p[32:64, :], VB(f"xw{kb}")[:, 16:48], nsl(xc_t[kb], n),
                                 start=(kb == 0), stop=(kb == 3))
            nc.vector.tensor_copy(nsl(dt_sb, n), p[0:16, :])
            nc.vector.tensor_copy(nsl(bc_sb, n), p[32:64, :])

        # dt_proj -> softplus -> delta (bf16; scan's a is computed in f32 by ACT)
        delta_t = []
        for db in range(4):
            d = sb.tile([128, L], bf16, tag=f"dl{db}")
            for n in range(NT):
                p = psA.tile([128, 512], f32, tag="mm", name="p_dt")
                nc.tensor.matmul(p[:], VB("dtw")[:, db * 128:(db + 1) * 128],
                                 nsl(dt_sb, n), start=True, stop=True)
                se = scr.tile([128, 512], f32, tag="se", bufs=1, name="se")
                nc.scalar.activation(se[:], p[:], AF.Exp,
                                     bias=VF(f"dtb{db}")[:, 0:1], scale=1.0)
                nc.scalar.activation(nsl(d, n), se[:], AF.Ln, bias=1.0, scale=1.0)
            delta_t.append(d)

        # c = delta * xc (bf16)
        c_t = []
        for db in range(4):
            c = sb.tile([128, L], bf16, tag=f"c{db}")
            nc.vector.tensor_mul(c[:], delta_t[db][:], xc_t[db][:])
            c_t.append(c)

        # selective scan, s-major; y accumulated in bf16
        # B/C row broadcasts via DMA from DRAM (free engines)
        bc_d = dram.tile([16, 2 * L], bf16, tag="bc_d")
        nc.sync.dma_start(out=bc_d[:, 0:L], in_=bc_sb[0:16, :])
        nc.sync.dma_start(out=bc_d[:, L:2 * L], in_=bc_sb[16:32, :])
        y_t = [sb.tile([128, L], bf16, tag=f"y{db}", name=f"y{db}") for db in range(4)]
        for s in range(DS):
            bbcc = scr.tile([128, 2 * L], bf16, tag="bbcc", name="bbcc")
            nc.sync.dma_start(out=bbcc[:],
                              in_=bc_d[s:s + 1, :].broadcast_to([128, 2 * L]))
            bbs, ccs = bbcc[:, 0:L], bbcc[:, L:2 * L]
            for db in range(4):
                a = scr.tile([128, L], bf16, tag="a_s", name="a_s")
                nc.scalar.activation(a[:], delta_t[db][:], AF.Exp,
                                     bias=0.0, scale=-float(s + 1))
                bv = scr.tile([128, L], bf16, tag="bv", name="bv")
                nc.gpsimd.tensor_mul(bv[:], c_t[db][:], bbs)
                h = scr.tile([128, L], bf16, tag="h_s", name="h_s")
                nc.vector.tensor_tensor_scan(h[:], a[:], bv[:], 0.0,
                                             op0=OP.mult, op1=OP.add)
                if s == 0:
                    nc.vector.tensor_mul(y_t[db][:], h[:], ccs)
                else:
                    hc = scr.tile([128, L], bf16, tag="hc", name="hc")
                    nc.vector.tensor_mul(hc[:], h[:], ccs)
                    nc.gpsimd.tensor_add(y_t[db][:], y_t[db][:], hc[:])

        # y = (y + Dp*xc) * silu(z), in place; out_proj reads z_t
        yg_t = []
        for db in range(4):
            nc.vector.scalar_tensor_tensor(y_t[db][:], xc_t[db][:], VF(f"dp{db}")[:, 0:1],
                                           y_t[db][:], op0=OP.mult, op1=OP.add)
            nc.vector.tensor_mul(z_t[db][:], y_t[db][:], z_t[db][:])
            yg_t.append(z_t[db])

        accm = []
        for pb in range(2):
            t = sb.tile([128, L], bf16, tag=f"accm{pb}")
            for n in range(NT):
                p = psA.tile([128, 512], f32, tag="mm", name="p_op")
                for kb in range(4):
                    nc.tensor.matmul(p[:], VB(f"ow{kb}")[:, pb * 128:(pb + 1) * 128],
                                     nsl(yg_t[kb], n), start=(kb == 0), stop=(kb == 3))
                nc.scalar.activation(nsl(t, n), p[:], AF.Copy, bias=0.0, scale=1.0)
            accm.append(t)

        # ---------------- layer norm helper (channels on partitions, 2 pb tiles)
        def layer_norm(src2, idx, dst_aps, lnexp=False):
            """src2: 2 tiles (128, F). dst_aps: fn(pb, n) -> out AP (128, 512)."""
            F = src2[0].shape[1]
            for n in range(F // 512):
                m1 = psA.tile([128, 512], f32, tag="mm", name="ln_m1")
                for pb in range(2):
                    nc.tensor.matmul(m1[0:1, :], VB("ones_col"), nsl(src2[pb], n),
                                     start=(pb == 0), stop=(pb == 1))
                m2 = psA.tile([128, 512], f32, tag="mm", name="ln_m2")
                for pb in range(2):
                    sq = scr.tile([128, 512], bf16, tag="ln_sq", name="ln_sq")
                    nc.scalar.activation(sq[:], nsl(src2[pb], n), AF.Square,
                                         bias=0.0, scale=1.0)
                    nc.tensor.matmul(m2[0:1, :], VB("ones_col"), sq[:],
                                     start=(pb == 0), stop=(pb == 1))
                mean = scr.tile([1, 512], f32, tag="ln_sm", bufs=4, name="mean")
                nc.vector.tensor_scalar_mul(mean[:], m1[0:1, :], 1.0 / DM)
                var = scr.tile([1, 512], f32, tag="ln_sm", bufs=4, name="var")
                nc.vector.tensor_mul(var[:], mean[:], mean[:])
                nc.vector.scalar_tensor_tensor(var[:], m2[0:1, :], 1.0 / DM, var[:],
                                               op0=OP.mult, op1=OP.subtract)
                sd = scr.tile([1, 512], f32, tag="ln_sm", bufs=4, name="sd")
                rstd = scr.tile([1, 512], f32, tag="ln_sm", bufs=4, name="rstd")
                if lnexp:     # stay in the exp/ln act-table set (avoids sqrt load)
                    nc.scalar.activation(sd[:], var[:], AF.Ln, bias=eps_t[:, 0:1], scale=1.0)
                    nc.scalar.activation(rstd[:], sd[:], AF.Exp, bias=0.0, scale=-0.5)
                else:
                    nc.scalar.activation(sd[:], var[:], AF.Sqrt, bias=eps_t[:, 0:1], scale=1.0)
                    nc.vector.reciprocal(rstd[:], sd[:])
                mrs = scr.tile([1, 512], f32, tag="ln_sm", bufs=4, name="mrs")
                nc.vector.tensor_mul(mrs[:], mean[:], rstd[:])
                rb = psA.tile([128, 512], f32, tag="mm", name="ln_rb")
                nc.tensor.matmul(rb[:], VF("ones_row_f"), rstd[:], start=True, stop=True)
                mb2 = psA.tile([128, 512], f32, tag="mm", name="ln_mb")
                nc.tensor.matmul(mb2[:], VF("ones_row_f"), mrs[:], start=True, stop=True)
                for pb in range(2):
                    t1 = scr.tile([128, 512], bf16, tag="ln_t1", name="ln_t1")
                    nc.vector.tensor_mul(t1[:], nsl(src2[pb], n), rb[:])
                    nc.vector.tensor_sub(t1[:], t1[:], mb2[:])
                    nc.scalar.activation(dst_aps(pb, n), t1[:], AF.Identity,
                                         bias=VF(f"lnb{idx}{pb}")[:, 0:1],
                                         scale=VF(f"lnw{idx}{pb}")[:, 0:1])


        # acc LN in place -> a2a_in
        accn = accm
        layer_norm(accm, 1, lambda pb, n: nsl(accm[pb], n))
        a2a_in = dram.tile([B, 256, SL], bf16, tag="a2a_in")
        a2a_out = dram.tile([B, 256, SL], bf16, tag="a2a_out")
        for pb in range(2):
            nc.sync.dma_start(
                out=a2a_in[:, pb * 128:(pb + 1) * 128, :].rearrange("u p n -> p u n"),
                in_=accn[pb][:].rearrange("p (u n) -> p u n", u=8))
        from concourse import mybir as _mb
        nc.gpsimd.collective_compute(
            "AllToAll", OP.bypass,
            replica_groups=[list(range(B))],
            ins=[a2a_in.opt()], outs=[a2a_out.opt()])

        # ---------------- phase B inputs (beta order: x, ang, acc)
        # hsrc_xa: x|ang (no exchange dep); hsrc_ac: exchanged acc branch
        hsrc_xa = [sb.tile([128, 2048], bf16, tag=f"accemb{pb}", name=f"hsrcxa{pb}")
                   for pb in range(2)]
        hsrc_ac = [sb.tile([128, 1024], bf16, tag=("accT" if pb == 0 else "angT"),
                           name=f"hsrcac{pb}") for pb in range(2)]

        # x branch: xs -> LN -> hsrc[:, 0:1024]
        xsb = []
        for pb in range(2):
            t = sb.tile([128, L], bf16, tag=f"xsb{pb}")
            nc.sync.dma_start(out=t[:], in_=xs_d[pb * 128:(pb + 1) * 128, :])
            xsb.append(t)
        layer_norm(xsb, 0, lambda pb, n: hsrc_xa[pb][:, n * 512:(n + 1) * 512])

        # ang branch: embed -> LN -> hsrc[:, 2048:3072]
        angT = sb.tile([12, L], bf16, tag="angT")
        nc.sync.dma_start(out=angT[:], in_=angT_d[:, :])
        ang_emb = []
        for pb in range(2):
            t = sb.tile([128, L], bf16, tag=f"angemb{pb}")
            for n in range(NT):
                p = psA.tile([128, 512], f32, tag="mm", name="p_ang")
                nc.tensor.matmul(p[:], VB("ang_wT")[:, pb * 128:(pb + 1) * 128],
                                 nsl(angT, n), start=True, stop=True)
                nc.scalar.activation(nsl(t, n), p[:], AF.Identity,
                                     bias=VF(f"ang_b{pb}")[:, 0:1], scale=1.0)
            ang_emb.append(t)
        layer_norm(ang_emb, 2,
                   lambda pb, n: hsrc_xa[pb][:, 1024 + n * 512:1024 + (n + 1) * 512])

        if debug:
            dt_ = scr.tile([128, L], f32, tag="dbgc", bufs=1, name="dbgc")
            nc.vector.tensor_copy(dt_[:], accn[0][:])
            nc.sync.dma_start(out=dbg_d[0, :, :], in_=dt_[:])
            dt2_ = scr.tile([128, L], f32, tag="dbgc", bufs=1, name="dbgc2")
            nc.vector.tensor_copy(dt2_[:], hsrc[0][:, 1024:2048])
            nc.sync.dma_start(out=dbg_d[1, :, :], in_=dt2_[:])

        # ---------------- QKV (reuse xz tags: xi dead after conv, z after yg)
        qkv_xa = [sb.tile([128, 2048], bf16, tag=f"xz{mb}", name=f"qkvxa{mb}")
                  for mb in range(6)]
        _qa_tags = ["qa0", "qa1", "qa2", "qa3", "xz6", "xz7"]
        qkv_ac = [sb.tile([128, 1024], bf16, tag=_qa_tags[mb], name=f"qkvac{mb}")
                  for mb in range(6)]

        def emit_qkv(dst, srcs, n_lo, n_hi, on_dve=False):
            for mb in range(6):
                for n in range(n_lo, n_hi):
                    p = psA.tile([128, 512], f32, tag="mm", name="p_qkv")
                    for kb in range(2):
                        nc.tensor.matmul(p[:], VB(f"aiw{kb}")[:, mb * 128:(mb + 1) * 128],
                                         nsl(srcs[kb], n), start=(kb == 0), stop=(kb == 1))
                    if on_dve:
                        nc.vector.tensor_scalar_add(nsl(dst[mb], n - n_lo), p[:],
                                                    VF(f"aib{mb}")[:, 0:1])
                    else:
                        nc.scalar.activation(nsl(dst[mb], n - n_lo), p[:], AF.Identity,
                                             bias=VF(f"aib{mb}")[:, 0:1], scale=1.0)

        emit_qkv(qkv_xa, hsrc_xa, 0, 4)     # x + ang: independent of the exchange

        def attn_group(tag, qkv_g, nb, beta_lo, s_range):
            """Attention for one branch group. nb = #branches (2 or 1).
            qkv_g tiles are (128, nb*1024): (beta, b, n) with b inside."""
            W = nb * SL          # 256 or 128
            FW = 8 * W           # full t-packed width (2048 or 1024)
            NCH = FW // 1024     # psum chunks of 1024

            def gsl(t_, b):
                if nb == 1:
                    return t_[:].rearrange("p (u n) -> p u n", u=8)[:, b, :]
                return t_[:].rearrange("p (g u n) -> p g u n", g=nb, u=8)[:, :, b, :]

            def tview(t_):
                # (128, 8t, nb, 128) iteration view of a (128, FW) t-major tile
                if nb == 1:
                    return t_[:].rearrange("p (u n) -> p u n", u=8)
                return t_[:].rearrange("p (u g n) -> p u g n", u=8, g=nb)

            def kview(t_):
                # K/V tile (128, nb*1024) iterated (t, beta, n)
                if nb == 1:
                    return t_[:].rearrange("p (u n) -> p u n", u=8)
                return t_[:].rearrange("p (g u n) -> p u g n", g=nb, u=8)

            for s in s_range:
                # scores: broadcast TT per (pb, 1024-chunk); head-reduce; packed exp
                TPC = 1024 // W           # t's per 1024 chunk
                E = scr.tile([8, FW + W], bf16, tag=f"E{tag}", bufs=2, name=f"E{tag}")
                for ch in range(NCH):
                    prods = []
                    for pb in range(2):
                        pr = scr.tile([128, 1024], bf16, tag=f"prod{pb}", bufs=2,
                                      name=f"prod{pb}")
                        q = gsl(qkv_g[pb], s)
                        qb = q.unsqueeze(1).broadcast_to(
                            [128, TPC] + list(q.shape[1:]))
                        kv_ = kview(qkv_g[2 + pb])[:, ch * TPC:(ch + 1) * TPC]
                        if nb == 1:
                            prv = pr[:].rearrange("p (u n) -> p u n", u=TPC)
                        else:
                            prv = pr[:].rearrange("p (u g n) -> p u g n",
                                                  u=TPC, g=nb)
                        nc.gpsimd.tensor_tensor(out=prv, in0=qb, in1=kv_,
                                                op=OP.mult)
                        prods.append(pr)
                    S = psS.tile([128, 1024], f32, tag="ps1024", name=f"S{tag}")
                    for t in range(TPC):
                        for pb in range(2):
                            nc.tensor.matmul(S[0:8, t * W:(t + 1) * W],
                                             VB(f"hsel{pb}"),
                                             prods[pb][:, t * W:(t + 1) * W],
                                             start=(pb == 0), stop=(pb == 1))
                    nc.scalar.activation(E[:, ch * 1024:(ch + 1) * 1024],
                                         S[0:8, :], AF.Exp, bias=0.0, scale=1.0)
                # denominator: tree over the 8 t-slices of E
                dd = []
                for i in range(4):
                    d_ = scr.tile([8, W], bf16, tag=f"dd{tag}{i}", bufs=1,
                                  name=f"dd{tag}{i}")
                    nc.vector.tensor_add(d_[:], E[:, 2 * i * W:(2 * i + 1) * W],
                                         E[:, (2 * i + 1) * W:(2 * i + 2) * W])
                    dd.append(d_)
                nc.vector.tensor_add(dd[0][:], dd[0][:], dd[1][:])
                nc.vector.tensor_add(dd[2][:], dd[2][:], dd[3][:])
                nc.vector.tensor_add(dd[0][:], dd[0][:], dd[2][:])
                R = E[:, FW:FW + W]
                with nc.allow_low_precision(reason="softmax recip bf16"):
                    nc.vector.reciprocal(R, dd[0][:])
                E_d = dram.tile([8, FW + W], bf16, tag=f"Ed{tag}", bufs=2,
                                name=f"Ed{tag}")
                nc.sync.dma_start(out=E_d[:], in_=E[:])
                Oacc = [None, None]
                ebcs = []
                for pb in range(2):
                    # one DMA: all 8 t-chunks of att plus 1/D, heads->channels
                    ebc = scr.tile([128, FW + W], bf16, tag=f"ebc{pb}", bufs=2,
                                   name=f"ebc{pb}")
                    (nc.sync if nb == 2 else nc.gpsimd).dma_start(
                        out=ebc[:],
                        in_=E_d[4 * pb:4 * pb + 4, :]
                        .unsqueeze(1).broadcast_to([4, 32, FW + W]))
                    ebcs.append(ebc)
                for pb in range(2):
                    ebc = ebcs[pb]
                    tsum = []
                    for ch in range(NCH):
                        tmp = scr.tile([128, 1024], bf16, tag="otmp", bufs=3,
                                       name="otmp")
                        vv = qkv_g[4 + pb]
                        if nb == 1:
                            vvw = vv[:, ch * 1024:(ch + 1) * 1024]
                            nc.vector.tensor_mul(tmp[:], ebc[:, ch * 1024:(ch + 1) * 1024], vvw)
                        else:
                            vvw = kview(vv)[:, 4 * ch:4 * ch + 4, :, :]
                            nc.vector.tensor_tensor(
                                out=tmp[:].rearrange("p (u g n) -> p u g n",
                                                     u=4, g=nb),
                                in0=ebc[:, ch * 1024:(ch + 1) * 1024]
                                .rearrange("p (u g n) -> p u g n", u=4, g=nb),
                                in1=vvw, op=OP.mult)
                        tsum.append(tmp)
                    # sum over t via PE identity-matmul accumulation in PSUM
                    t_slices = [tsum[i // (1024 // W)][:, (i % (1024 // W)) * W:
                                                       (i % (1024 // W) + 1) * W]
                                for i in range(8)]
                    osum = psA.tile([128, 512], f32, tag="mm", name=f"osum{tag}")
                    for i in range(8):
                        nc.tensor.matmul(osum[:, 0:W], VB("ident"), t_slices[i],
                                         start=(i == 0), stop=(i == 7))
                    o_ = scr.tile([128, W], bf16, tag=f"Oacc{tag}{pb}", bufs=2,
                                  name=f"Oacc{tag}{pb}")
                    # normalize by 1/denominator while copying out of PSUM
                    nc.vector.tensor_mul(o_[:], osum[:, 0:W],
                                         ebcs[pb][:, FW:FW + W])
                    Oacc[pb] = o_
                # out_proj + bias + store
                for mb in range(2):
                    p = psA.tile([128, 512], f32, tag="mm", name=f"pao{tag}")
                    for kb in range(2):
                        nc.tensor.matmul(p[:, 0:W], VB(f"aow{kb}")[:, mb * 128:(mb + 1) * 128],
                                         Oacc[kb][:], start=(kb == 0), stop=(kb == 1))
                    op = scr.tile([128, W], f32, tag=f"oproj{tag}", bufs=2,
                                  name=f"oproj{tag}")
                    nc.scalar.activation(op[:], p[:, 0:W], AF.Identity,
                                         bias=VF(f"aob{mb}")[:, 0:1], scale=1.0)
                    if nb == 1:
                        nc.sync.dma_start(
                            out=out_d[mb * 128:(mb + 1) * 128, s, beta_lo, :],
                            in_=op[:])
                    else:
                        nc.sync.dma_start(
                            out=out_d[mb * 128:(mb + 1) * 128, s,
                                      beta_lo:beta_lo + nb, :],
                            in_=op[:].rearrange("p (g n) -> p g n", g=nb))

        # acc branch first in emission order: it is the post-exchange critical
        # path, so it must outrank the x+ang group once the collective lands
        for pb in range(2):
            nc.sync.dma_start(
                out=hsrc_ac[pb][:].rearrange("p (u n) -> p u n", u=8),
                in_=a2a_out[:, pb * 128:(pb + 1) * 128, :].rearrange("u p n -> p u n"))
        emit_qkv(qkv_ac, hsrc_ac, 0, 2)
        # interleave acc/x+ang s-iterations in priority order: acc s leads
        attn_group("x", qkv_xa, 2, 0, range(0, 1))
        for s in range(8):
            if s < 7:
                attn_group("x", qkv_xa, 2, 0, range(s + 1, s + 2))
            attn_group("a", qkv_ac, 1, 2, range(s, s + 1))

    nc.finalize()
    return nc


def _get_nc(debug=False):
    key = "ncd" if debug else "nc"
    nc = _NC_CACHE.get(key)
    if nc is None:
        nc = _build(debug=debug)
        _NC_CACHE[key] = nc
    return nc


# ---------------------------------------------------------------- host wrapper
def _prep_in_maps(inp):
    wb, wb2, wf = _pack_weights(inp)
    x = np.asarray(inp["x"], np.float32)
    accele = np.asarray(inp["accele"], np.float32)
    angle = np.asarray(inp["angle"], np.float32)
    in_maps = []
    for c in range(B):
        sl = slice(c * SL, (c + 1) * SL)
        in_maps.append({
            "wb": wb, "wb2": wb2, "wf": wf,
            "accT": np.ascontiguousarray(accele[c].T).astype(BF),
            "angT": np.ascontiguousarray(
                angle[:, sl, :].transpose(2, 0, 1).reshape(12, L)).astype(BF),
            "xs": np.ascontiguousarray(
                x[:, sl, :].transpose(2, 0, 1).reshape(256, L)).astype(BF),
        })
    return in_maps


def _assemble(results):
    # per-core out: (256, B, 3, SL) -> final (B, L, 3*DM)
    out = np.zeros((B, L, 3 * DM), np.float32)
    chmap = {0: 0, 1: 2, 2: 1}        # device beta (x, ang, acc) -> output block
    for c in range(B):
        o = results[c]["out"]          # (256ch, 8b, 3beta, 128n)
        for beta in range(3):
            blk = chmap[beta]
            out[:, c * SL:(c + 1) * SL, blk * DM:(blk + 1) * DM] = \
                o[:, :, beta, :].transpose(1, 2, 0)
    return out


def run_hw(inp, debug=False):
    from concourse.bass_utils import run_bass_kernel_spmd
    nc = _get_nc(debug=debug)
    res = run_bass_kernel_spmd(nc, _prep_in_maps(inp), core_ids=list(range(B)))
    return _assemble(res.results), res


# ------------------------------------------------------------------ numpy fallback
def _ln_np(x, w, b):
    m = x.mean(-1, keepdims=True)
    v = ((x - m) ** 2).mean(-1, keepdims=True)
    return (x - m) / np.sqrt(v + 1e-5) * w + b


def _silu_np(x):
    return x / (1.0 + np.exp(-x))


def _mamba_np(x, in_w, conv_w, conv_b, x_proj_w, dt_w, dt_b, A_log, Dp, out_w):
    xz = x @ in_w.T
    xi, z = xz[:, :DI], xz[:, DI:]
    xpad = np.concatenate([np.zeros((DC - 1, DI), np.float32), xi], axis=0)
    w = conv_w[:, 0, :]
    xc = np.zeros_like(xi)
    for j in range(DC):
        xc += xpad[j:j + L] * w[:, j]
    xc = _silu_np(xc + conv_b)
    dbl = xc @ x_proj_w.T
    dt, Bm, Cm = dbl[:, :DTR], dbl[:, DTR:DTR + DS], dbl[:, DTR + DS:]
    delta = np.log1p(np.exp(dt @ dt_w.T + dt_b))
    A = -np.exp(A_log)
    h = np.zeros((DI, DS), np.float32)
    ys = np.zeros((L, DI), np.float32)
    for t in range(L):
        h = h * np.exp(delta[t][:, None] * A) + (delta[t] * xc[t])[:, None] * Bm[t][None, :]
        ys[t] = h @ Cm[t]
    y = ys + xc * Dp
    return (y * _silu_np(z)) @ out_w.T


def _phase2_np(h_pre, attn_in_w, attn_in_b, attn_out_w, attn_out_b):
    E = DM
    qkv = h_pre @ attn_in_w.T + attn_in_b
    q, k, v = qkv[..., :E], qkv[..., E:2 * E], qkv[..., 2 * E:]
    rs = lambda t: t.reshape(B, 3 * L, NH, DH)
    q = rs(q) / np.float32(np.sqrt(DH))
    k, v = rs(k), rs(v)
    att = np.einsum("snhd,tnhd->nhst", q, k)
    att = np.exp(att - att.max(axis=-1, keepdims=True))
    att = att / att.sum(axis=-1, keepdims=True)
    o = np.einsum("nhst,tnhd->snhd", att, v).reshape(B, 3 * L, E)
    return o @ attn_out_w.T + attn_out_b


def _kernel_numpy(inp):
    acc = inp["accele"] @ inp["acc_w"].T + inp["acc_b"]
    ang = inp["angle"] @ inp["ang_w"].T + inp["ang_b"]
    acc_m = np.stack([
        _mamba_np(acc[b], inp["in_proj_w"], inp["conv_w"], inp["conv_b"],
                  inp["x_proj_w"], inp["dt_proj_w"], inp["dt_proj_b"],
                  inp["A_log"], inp["Dp"], inp["out_proj_w"]) for b in range(B)])
    xn = _ln_np(inp["x"], inp["norm_w"], inp["norm_b"])
    accn = _ln_np(acc_m, inp["norm_acc_w"], inp["norm_acc_b"])
    angn = _ln_np(ang, inp["norm_ang_w"], inp["norm_ang_b"])
    h_pre = np.concatenate([xn, accn, angn], axis=1)
    h = _phase2_np(h_pre, inp["attn_in_w"], inp["attn_in_b"],
                   inp["attn_out_w"], inp["attn_out_b"])
    return np.concatenate([h[:, :L], h[:, L:2 * L], h[:, 2 * L:]],
                          axis=2).astype(np.float32)


USE_HW = True


def kernel(**inputs):
    inp = {k: np.asarray(v, dtype=np.float32) for k, v in inputs.items()}
    # the HW scan bakes A[d,s] = -(s+1) into activation scales; guard it
    a_ok = np.allclose(-np.exp(inp["A_log"]),
                       -np.arange(1, DS + 1, dtype=np.float32)[None, :].repeat(DI, 0),
                       rtol=1e-5)
    if USE_HW and a_ok:
        try:
            out, _ = run_hw(inp)
            return out
        except Exception:
            import traceback
            traceback.print_exc()
    return _kernel_numpy(inp)

